# revision 1
# baseline (speedup 1.0000x reference)
"""Multi-head causal attention (B=4, T=2048, D=1024, H=16) on 8 TRN2 NeuronCores.

Sharding: data-parallel over batch (4) x tensor-parallel over heads (2 groups
of 8). Core c handles batch c//2, head-group c%2. Each core computes its
Q/K/V projections (weight-column shards), causal attention for its 8 heads,
and a partial output projection (weight-row shard). The pairwise reduction of
the two partials per batch happens on host (cheap: 4 x 8MB adds).

All matmuls run as float32r (full PE rate at moving-dim >= 256, ~1.5e-4 rel
err vs fp32). Softmax skips the max-subtraction: scores are bounded (~|2|)
for any plausibly-scaled input, which exp() handles comfortably in fp32.
"""

import sys

if "/opt/trn_rl_repo" not in sys.path:
    sys.path.insert(0, "/opt/trn_rl_repo")

import numpy as np

import concourse.bass as bass
import concourse.mybir as mybir
from concourse import bacc
from concourse.bass import MemorySpace
from concourse.tile import TileContext

B, T, D = 4, 2048, 1024
H, DH = 16, 64
HG = 8          # heads per core (group)
GW = HG * DH    # group width = 512
N_CORES = 8
P = 128
NCH = T // 512  # 4 query chunks of 512
NTB = T // P    # 16 t-blocks of 128

F32 = mybir.dt.float32
F32R = mybir.dt.float32r




def build_nc():
    nc = bacc.Bacc()

    xq = nc.dram_tensor("xq", [D, T], F32R, kind="ExternalInput")
    xk = nc.dram_tensor("xk", [D, T], F32R, kind="ExternalInput")
    xv = nc.dram_tensor("xv", [D, T], F32R, kind="ExternalInput")
    wq = nc.dram_tensor("wq", [D, GW], F32R, kind="ExternalInput")
    wk = nc.dram_tensor("wk", [D, GW], F32R, kind="ExternalInput")
    wv = nc.dram_tensor("wv", [D, GW], F32R, kind="ExternalInput")
    wo = nc.dram_tensor("wo", [GW, D], F32R, kind="ExternalInput")
    masks = nc.dram_tensor("masks", [P, 4, 512], F32R, kind="ExternalInput")
    e12 = nc.dram_tensor("e12", [1, 256], F32R, kind="ExternalInput")
    out = nc.dram_tensor("out", [T, D], F32, kind="ExternalOutput")

    KD = D // P  # 8 contraction chunks for the projections

    with TileContext(nc) as tc:
        with (
            tc.tile_pool(name="big", bufs=1) as big,
            tc.tile_pool(name="qka", bufs=8) as qka,   # kt0-3 + qt0-3 (qt doubles as attn-out)
            tc.tile_pool(name="consts", bufs=1) as consts,
        ):
            _psA = tc.tile_pool(name="psum", bufs=2, space=MemorySpace.PSUM)
            psum = _psA.__enter__()

            vsb = big.tile([P, NTB, HG * 65], F32R, name="vsb")  # V aug: per head 65 cols (64 V + ones)
            mask_sb = big.tile([P, 4, 512], F32R, name="mask_sb")
            e12_sb = consts.tile([1, 256], F32R, name="e12_sb")
            nc.sync.dma_start(mask_sb, masks[:, :, :])
            nc.sync.dma_start(e12_sb, e12[:, :])
            # ones column of each head slot (f32r memset fails the ISA check -> uint32 bit pattern)
            vones = vsb.rearrange("p tb (h m) -> p tb h m", h=HG)[:, :, :, 64:65]
            nc.vector.memset(vones.bitcast(mybir.dt.uint32), 0x3F800000)

            lo, hi = slice(0, 64), slice(64, 128)

            _pp = tc.tile_pool(name="ppool", bufs=8)
            ppool = _pp.__enter__()
            _rp = tc.tile_pool(name="rpool", bufs=2)
            rpool = _rp.__enter__()
            _xw = tc.tile_pool(name="xwpool", bufs=2)
            xwpool = _xw.__enter__()

            # ---- K projection (x streamed once; 256-wide chunks) ----
            kts = [qka.tile([P, T], F32R, name=f"kt{j}", tag="qka") for j in range(4)]
            qts = [qka.tile([P, T], F32R, name=f"qt{j}", tag="qka") for j in range(4)]
            wk_sb = xwpool.tile([P, KD, GW], F32R, name="wk_sb", tag="wfull")
            nc.sync.dma_start(wk_sb, wk.rearrange("(ko p) j -> p ko j", p=P))
            for ch in range(8):
                xt = xwpool.tile([P, KD, 256], F32R, name="xt", tag="xs")
                nc.sync.dma_start(
                    xt, xk.rearrange("(ko p) t -> p ko t", p=P)[:, :, ch * 256:(ch + 1) * 256]
                )
                for jb in range(4):
                    ps = psum.tile([P, 256], F32, name="ps_kq", tag="ps")
                    for kd in range(KD):
                        nc.tensor.matmul(
                            ps, wk_sb[:, kd, jb * P:(jb + 1) * P], xt[:, kd, :],
                            start=(kd == 0), stop=(kd == KD - 1),
                        )
                    nc.vector.tensor_copy(kts[jb][:, ch * 256:(ch + 1) * 256], ps)

            # ---- Q projection: chunks 0-3 inline; chunks 4-7 paced into attention ----
            wq_sb = xwpool.tile([P, KD, GW], F32R, name="wq_sb", tag="wfull")
            nc.sync.dma_start(wq_sb, wq.rearrange("(ko p) j -> p ko j", p=P))
            qsteps = []
            xts_q = {}

            def queue_qchunk(ch, inline):
                if inline:
                    xt = xwpool.tile([P, KD, 256], F32R, name="xt", tag="xs")
                    nc.sync.dma_start(
                        xt, xq.rearrange("(ko p) t -> p ko t", p=P)[:, :, ch * 256:(ch + 1) * 256]
                    )
                    xts_q[ch] = xt
                    for jb in range(4):
                        ps = psum.tile([P, 256], F32, name="ps_q", tag="ps")
                        for kd in range(KD):
                            nc.tensor.matmul(
                                ps, wq_sb[:, kd, jb * P:(jb + 1) * P], xt[:, kd, :],
                                start=(kd == 0), stop=(kd == KD - 1),
                            )
                        nc.vector.tensor_copy(qts[jb][:, ch * 256:(ch + 1) * 256], ps)
                    return

                def dma_step(ch=ch):
                    xt = xwpool.tile([P, KD, 256], F32R, name="xt", tag="xs")
                    nc.sync.dma_start(
                        xt, xq.rearrange("(ko p) t -> p ko t", p=P)[:, :, ch * 256:(ch + 1) * 256]
                    )
                    xts_q[ch] = xt

                if ch == 4:
                    qsteps.append((-1, lambda: dma_step(4)))
                    qsteps.append((-1, lambda: dma_step(5)))
                elif ch < 7:
                    qsteps.append((-1, lambda ch=ch: dma_step(ch + 1)))
                for jb in range(4):
                    box = {}

                    def step(kd, jb=jb, ch=ch, box=box):
                        if kd == 0:
                            box["ps"] = psum.tile([P, 256], F32, name="ps_q", tag="ps")
                        nc.tensor.matmul(
                            box["ps"], wq_sb[:, kd, jb * P:(jb + 1) * P], xts_q[ch][:, kd, :],
                            start=(kd == 0), stop=(kd == KD - 1),
                        )
                        if kd == KD - 1:
                            nc.vector.tensor_copy(
                                qts[jb][:, ch * 256:(ch + 1) * 256], box["ps"]
                            )

                    for kd in range(KD):
                        qsteps.append((ch * 4 + jb, lambda kd=kd, step=step: step(kd)))

            def drain_qsteps(n):
                for _ in range(n):
                    if qsteps:
                        qsteps.pop(0)[1]()

            def drain_until(key):
                # emit every queued Q step needed for (chunk, pair) <= key
                while qsteps and qsteps[0][0] <= key:
                    qsteps.pop(0)[1]()

            for ch in range(4):
                queue_qchunk(ch, inline=True)
            for ch in range(4, 8):
                queue_qchunk(ch, inline=False)

            # ---- V projection (128-wide t-blocks straight into vsb) ----
            wv_sb = xwpool.tile([P, KD, GW], F32R, name="wv_sb", tag="wfull")
            nc.sync.dma_start(wv_sb, wv.rearrange("(ko p) j -> p ko j", p=P))
            for ch in range(16):
                xt = xwpool.tile([P, KD, 128], F32R, name="xt", tag="xs")
                nc.sync.dma_start(
                    xt, xv.rearrange("(ko p) t -> p ko t", p=P)[:, :, ch * 128:(ch + 1) * 128]
                )
                ps = psum.tile([P, 512], F32, name="ps_v", tag="ps")
                for kd in range(KD):
                    nc.tensor.matmul(
                        ps, xt[:, kd, :], wv_sb[:, kd, :],
                        start=(kd == 0), stop=(kd == KD - 1),
                    )
                nc.vector.tensor_copy(
                    vsb[:, ch, :].rearrange("p (h m) -> p h m", h=HG)[:, :, 0:64],
                    ps.rearrange("p (h m) -> p h m", h=HG),
                )

            # ---- attention (pure pipeline; AV emission lags 2 units) ----
            aots = []
            for pr in range(4):
                if pr >= 2:
                    drain_qsteps(999)
                kt = qt = None
                kt, qt = kts[pr], qts[pr]
                # attention output reuses qt's storage: qt[:, chunk] is dead
                # after that chunk's QK^T matmuls, exactly when normalize writes it
                aot = qt
                aots.append(aot)

                def emit_av(u):
                    (uc, ublk, up, ufirst, ulast) = u
                    if ufirst:
                        av1_t[uc] = psum.tile([65, 512], F32, name="av1", tag="av")
                        av2_t[uc] = psum.tile([65, 512], F32, name="av2", tag="av")
                    nc.tensor.matmul(
                        av1_t[uc], vsb[:, ublk, (2 * pr) * 65:(2 * pr) * 65 + 65],
                        up[:, 0:512], start=ufirst, stop=ulast,
                    )
                    nc.tensor.matmul(
                        av2_t[uc], vsb[:, ublk, (2 * pr + 1) * 65:(2 * pr + 1) * 65 + 65],
                        up[:, 512:1024], start=ufirst, stop=ulast,
                    )

                def emit_tail(uc):
                    # denominators -> reciprocal -> broadcast -> normalize
                    # (reciprocal_approx_fast silently no-ops at base partition != 0)
                    av1, av2 = av1_t[uc], av2_t[uc]
                    ucs = slice(uc * 512, (uc + 1) * 512)
                    rt = rpool.tile([1, 1024], F32, name="rt", tag="rt", bufs=1)
                    nc.vector.tensor_copy(rt[0:1, 0:512], av1[64:65, :])
                    nc.vector.tensor_copy(rt[0:1, 512:1024], av2[64:65, :])
                    nc.vector.reciprocal_approx_fast(rt, rt)
                    rt_r = rpool.tile([1, 1024], F32R, name="rt_r", tag="rtr", bufs=1)
                    nc.vector.tensor_copy(rt_r, rt)
                    bc = psum.tile([P, 512], F32, name="bc", tag="ps")
                    nc.tensor.matmul(bc, e12_sb[:, 0:128], rt_r[:, 0:512], start=True, stop=False)
                    nc.tensor.matmul(bc, e12_sb[:, 128:256], rt_r[:, 512:1024], start=False, stop=True)
                    bcn = rpool.tile([P, 512], F32, name="bcn", tag="bcn", bufs=1)
                    nc.vector.tensor_copy(bcn, bc)
                    nc.vector.tensor_mul(aot[lo, ucs], av1[0:64, :], bcn[lo, :])
                    nc.vector.tensor_mul(aot[hi, ucs], av2[0:64, :], bcn[hi, :])

                av1_t, av2_t = {}, {}
                pend = []
                for c in range(NCH):
                    cs = slice(c * 512, (c + 1) * 512)
                    nblk = 4 * (c + 1)
                    drain_until((2 * c + 1) * 4 + pr)
                    for blk in range(nblk):
                        ks = slice(blk * P, (blk + 1) * P)
                        s_pair = psum.tile([P, 1024], F32, name="s_pair", tag="sp")
                        nc.tensor.matmul(
                            s_pair[:, 0:512], kt[lo, ks], qt[lo, cs], start=True, stop=True,
                        )
                        nc.tensor.matmul(
                            s_pair[:, 512:1024], kt[hi, ks], qt[hi, cs], start=True, stop=True,
                        )
                        p_pair = ppool.tile([P, 1024], F32R, name="p_pair", tag="pp")
                        nc.scalar.activation(
                            p_pair, s_pair, mybir.ActivationFunctionType.Exp,
                            scale=float(DH) ** -0.5,
                        )
                        di = blk - (nblk - 4)
                        if di >= 0:
                            nc.vector.tensor_mul(
                                p_pair[:, 0:512], p_pair[:, 0:512], mask_sb[:, di, :]
                            )
                            nc.gpsimd.tensor_mul(
                                p_pair[:, 512:1024], p_pair[:, 512:1024], mask_sb[:, di, :]
                            )
                        drain_qsteps(2)
                        pend.append((c, blk, p_pair, blk == 0, blk == nblk - 1))
                        if len(pend) > 5:
                            u = pend.pop(0)
                            emit_av(u)
                            if u[4]:
                                emit_tail(u[0])
                while pend:
                    u = pend.pop(0)
                    emit_av(u)
                    if u[4]:
                        emit_tail(u[0])

            _xw.__exit__(None, None, None)
            _rp.__exit__(None, None, None)
            _pp.__exit__(None, None, None)

            # ---- output projection ----
            with tc.tile_pool(name="opool", bufs=2) as opool:
                wo_sb = opool.tile([P, 4, D], F32R, name="wo_sb", tag="wo", bufs=1)
                nc.sync.dma_start(wo_sb, wo.rearrange("(jb p) o -> p jb o", p=P))
                for tb in range(NTB):
                    ob = opool.tile([P, D], F32, name="ob", tag="ob")
                    for oc in range(2):
                        ps = psum.tile([P, 512], F32, name="o_ps_t", tag="ps")
                        for jb in range(4):
                            nc.tensor.matmul(
                                ps,
                                aots[jb][:, tb * P:(tb + 1) * P],
                                wo_sb[:, jb, oc * 512:(oc + 1) * 512],
                                start=(jb == 0),
                                stop=(jb == 3),
                            )
                        nc.vector.tensor_copy(ob[:, oc * 512:(oc + 1) * 512], ps)
                    nc.sync.dma_start(out[tb * P:(tb + 1) * P, :], ob)
            _psA.__exit__(None, None, None)

    nc.finalize()
    return nc


def _host_consts():
    m = np.zeros((P, 4, 512), dtype=np.float32)
    for di, delta in enumerate((0, 128, 256, 384)):
        pv = np.arange(P)[:, None]
        fv = np.arange(512)[None, :]
        m[:, di, :] = (fv >= pv + delta).astype(np.float32)
    e = np.zeros((1, 256), dtype=np.float32)
    e[0, 0:64] = 1.0
    e[0, 192:256] = 1.0
    return m, e


_NC_CACHE = None


def make_in_maps(q, k, v, Wq, Wk, Wv, Wo):
    masks_h, e2b_h = _host_consts()
    in_maps = []
    for c in range(N_CORES):
        b, g = c // 2, c % 2
        hs = slice(g * GW, (g + 1) * GW)
        in_maps.append({
            "xq": np.ascontiguousarray(q[b].T),
            "xk": np.ascontiguousarray(k[b].T),
            "xv": np.ascontiguousarray(v[b].T),
            "wq": np.ascontiguousarray(Wq[hs, :].T),
            "wk": np.ascontiguousarray(Wk[hs, :].T),
            "wv": np.ascontiguousarray(Wv[hs, :].T),
            "wo": np.ascontiguousarray(Wo[:, hs].T),
            "masks": masks_h,
            "e12": e2b_h,
        })
    return in_maps


def kernel(q, k, v, mask, Wq, Wk, Wv, Wo):
    global _NC_CACHE
    if _NC_CACHE is None:
        _NC_CACHE = build_nc()
    nc = _NC_CACHE

    from concourse.bass_utils import run_bass_kernel_spmd

    q, k, v = np.asarray(q), np.asarray(k), np.asarray(v)
    Wq, Wk, Wv, Wo = (np.asarray(t) for t in (Wq, Wk, Wv, Wo))
    in_maps = make_in_maps(q, k, v, Wq, Wk, Wv, Wo)

    r = run_bass_kernel_spmd(nc, in_maps, core_ids=list(range(N_CORES)))
    parts = [r.results[c]["out"] for c in range(N_CORES)]
    y = np.stack([parts[2 * b] + parts[2 * b + 1] for b in range(B)]).astype(np.float32)
    return y

